# revision 31
# baseline (speedup 1.0000x reference)
"""Causal self-attention on 8 Trainium2 NeuronCores (Bass/Tile).

Problem: nn_CausalSelfAttention (B=4, T=2048, C=1024, H=16 heads, fp32).

Sharding: tensor-parallel over heads for QKV projection + attention
(2 heads per core), per-batch AllGather of attention outputs (fp16,
transposed layout), then tensor-parallel over output columns for the
final projection (each core computes a 128-column slice of x@W_proj).

Schedule: software-pipelined per batch.  The attention kt-stream of
batch b is interleaved with "PE filler" groups — the QKV projection
row-tiles of batch b+1 and the output-projection row-tiles of batch
b-1 — so the tensor engine never idles while the scalar engine works
through the exp() stream (exp is the per-batch critical path; PE idle
gaps would also drop the HAM clock gate to half rate).

Layouts (feature dim on partitions everywhere):
  xT      [C, B*T]        input (fp16), replicated to all cores
  Q^T,K^T [CH, B*T]       CH = 2 heads x 64, heads stacked on
                          partitions 0:64 / 64:128
  V       [B*T, CH]       matmul lhsT for P@V, stored tiled with an
                          extra ones-column per head so the PV matmul
                          also produces softmax denominators
  S^T     [kr, 2*q] pair  scores transposed, both heads side by side in
                          one 2-bank PSUM tile; the two score matmuls
                          are K=64 row-tiled pairs (tile_position
                          (0,0)/(64,0)) that execute concurrently; one
                          exp() activation covers both heads
  attn^T  [CH, T] fp16    per-core, per-batch -> AllGather -> [C, T]
  y^T     [OC, B*T]       per-core 128-column slice of the final output

Softmax: unnormalized exp (scores are O(1)); causal mask = PE add of a
-60000 upper-triangular [128,128] constant onto the diagonal strip;
denominator from the V ones-column; division via
reciprocal_approx_fast on the PE-broadcast denominator.
All matmuls run fp16 inputs with fp32 PSUM accumulation.
"""

import numpy as np
from contextlib import ExitStack

P = 128
NQ = 512  # q/moving-operand tile width
MASKVAL = -60000.0


def build_attention_nc(B, T, C, H, n_cores):
    import concourse.bass as bass  # noqa: F401
    import concourse.bacc as bacc
    import concourse.tile as tile
    import concourse.mybir as mybir

    f32 = mybir.dt.float32
    fp16 = mybir.dt.float16
    Exp = mybir.ActivationFunctionType.Exp

    hs = C // H              # head size
    hpc = H // n_cores       # heads per core
    CH = hpc * hs            # qkv channels per core
    OC = C // n_cores        # output columns per core
    NT = B * T               # tokens
    KT_E = C // P            # contraction tiles over embedding dim
    TQ = T // NQ             # q tiles per batch
    TK = T // P              # kr tiles per batch
    TKALL = NT // P          # kr tiles over all batches
    DPB = NQ // P            # kr-tiles crossing one q-tile's diagonal
    WV = hpc * (hs + 1)      # V storage width per kr-tile (with ones cols)

    assert T % NQ == 0 and C % P == 0 and NT % NQ == 0
    assert CH == P and H % n_cores == 0 and hpc == 2 and hs == 64
    scale = 1.0 / float(np.sqrt(hs))

    nc = bacc.Bacc("TRN2", target_bir_lowering=False, debug=False,
                   num_devices=n_cores)

    xT = nc.dram_tensor("xT", [C, NT], fp16, kind="ExternalInput")
    wqkv = nc.dram_tensor("wqkv", [C, 3 * CH], fp16, kind="ExternalInput")
    bqkv = nc.dram_tensor("bqkv", [CH, 3], f32, kind="ExternalInput")
    wp = nc.dram_tensor("wp", [C, OC], fp16, kind="ExternalInput")
    bp = nc.dram_tensor("bp", [OC, 1], f32, kind="ExternalInput")
    yT = nc.dram_tensor("yT", [OC, NT], f32, kind="ExternalOutput")

    ident_np = np.eye(P, dtype=np.float16)
    # mask[p, c] = MASKVAL where kr-offset p > q-offset c (strictly lower)
    mask_np = np.where(
        np.arange(P)[:, None] > np.arange(P)[None, :],
        np.float16(MASKVAL), np.float16(0.0)).astype(np.float16)
    ident_dram = nc.inline_tensor(ident_np, name="ident_const")
    mask_dram = nc.inline_tensor(mask_np, name="mask_const")
    ones_dram = nc.inline_tensor(np.ones((P, hs), dtype=np.float16),
                                 name="ones_const")
    # Vn image with the denominator ones-columns baked in; the value
    # columns are overwritten by the V transposes at runtime
    vinit_np = np.zeros((P, TKALL * WV), dtype=np.float16)
    vinit_np.reshape(P, TKALL, hpc, hs + 1)[:, :, :, hs] = 1.0
    vinit_dram = nc.inline_tensor(vinit_np, name="vinit_const")

    with tile.TileContext(nc) as tc, ExitStack() as ctx:
        const = ctx.enter_context(tc.tile_pool(name="const", bufs=1))
        big = ctx.enter_context(tc.tile_pool(name="big", bufs=1))
        xin = ctx.enter_context(tc.tile_pool(name="xin", bufs=6))
        evac = ctx.enter_context(tc.tile_pool(name="evac", bufs=3))
        pexp = ctx.enter_context(tc.tile_pool(name="pexp", bufs=5))
        stp = ctx.enter_context(tc.tile_pool(name="stp", bufs=3, space="PSUM"))
        pvp = ctx.enter_context(tc.tile_pool(name="pvp", bufs=2, space="PSUM"))
        dram = ctx.enter_context(tc.tile_pool(name="dram", bufs=1, space="DRAM"))

        ident_t = const.tile([P, P], fp16)
        mask_sb = const.tile([P, P], fp16)
        ones_sb = const.tile([P, hs], fp16)
        bqkv_sb = const.tile([CH, 3], f32)
        bp_sb = const.tile([OC, 1], f32)
        w_sb = const.tile([P, KT_E * 3 * CH], fp16)
        wp_sb = const.tile([P, KT_E * OC], fp16)

        nc.sync.dma_start(bqkv_sb[:], bqkv[:])
        nc.sync.dma_start(ident_t[:], ident_dram[:])
        nc.sync.dma_start(mask_sb[:], mask_dram[:])
        nc.sync.dma_start(ones_sb[:], ones_dram[:])
        nc.sync.dma_start(bp_sb[:], bp[:])
        # weights k-slice by k-slice so the first QKV matmul can start
        # ~1us after the first two transfers instead of after the full load
        for k in range(KT_E):
            nc.sync.dma_start(
                w_sb[:, k * 3 * CH:(k + 1) * 3 * CH],
                wqkv[k * P:(k + 1) * P, :],
            )

        QT = big.tile([P, NT], fp16)
        KTp = big.tile([P, NT], fp16)
        VT = big.tile([CH, NT], fp16)
        Vn = big.tile([P, TKALL * WV], fp16)

        def load_late_consts():
            # not needed until the attention-PV / projection phases
            # (ones columns of V via one contiguous DMA of the full Vn
            # image; value columns get overwritten later)
            nc.sync.dma_start(Vn[:], vinit_dram[:])
            nc.sync.dma_start(
                wp_sb[:].rearrange("p (k m) -> p k m", k=KT_E),
                wp[:].rearrange("(k p) m -> p k m", p=P),
            )

        # ---- QKV projection for one 512-token row-tile; emitted either
        # directly (batch 0) or as filler groups inside an attention batch
        def qkv_row_fillers(n):
            ns = n * NQ
            xt_box = []

            def load_x():
                xt = xin.tile([P, KT_E * NQ], fp16, tag="xcol")
                nc.sync.dma_start(
                    xt[:].rearrange("p (k q) -> p k q", k=KT_E),
                    xT[:, ns:ns + NQ].rearrange("(k p) q -> p k q", p=P),
                )
                xt_box.append(xt)

            def mm_group(m):
                def go():
                    xt = xt_box[0]
                    ps = stp.tile([P, 2 * NQ], f32, tag="st")
                    for k in range(KT_E):
                        nc.tensor.matmul(
                            ps[:, 0:NQ],
                            lhsT=w_sb[:, k * 3 * CH + m * CH:
                                      k * 3 * CH + (m + 1) * CH],
                            rhs=xt[:, k * NQ:(k + 1) * NQ],
                            start=(k == 0),
                            stop=(k == KT_E - 1),
                        )
                    dst = (QT, KTp, VT)[m]
                    nc.vector.tensor_scalar_add(dst[:, ns:ns + NQ],
                                                ps[:, 0:NQ],
                                                bqkv_sb[:, m:m + 1])
                return go

            def transposes():
                tp = stp.tile([P, DPB * CH], fp16, tag="st", name="tp")
                for j in range(DPB):
                    nc.tensor.transpose(
                        tp[:, j * CH:(j + 1) * CH],
                        VT[:, (n * DPB + j) * P:(n * DPB + j + 1) * P],
                        ident_t[:],
                    )
                vi0 = n * DPB
                dst = Vn[:, vi0 * WV:(vi0 + DPB) * WV].rearrange(
                    "p (v h d) -> p v h d", h=hpc, d=hs + 1
                )[:, :, :, 0:hs]
                nc.vector.tensor_copy(dst, tp[:].rearrange(
                    "p (v h d) -> p v h d", h=hpc, d=hs))

            load_x()
            return [mm_group(0), mm_group(1), mm_group(2), transposes]

        # ---- per-batch AllGather pieces (token ranges): every batch is
        # gathered in two halves so the projection rows unblock early; the
        # last batch's tail is split ever finer so the final AllGather ->
        # projection chain is as short as possible
        pieces = []  # per batch: list of (tok_start, tok_len)
        for b in range(B):
            if b == B - 1:
                pieces.append([(0, 1024), (1024, 512),
                               (1536, 256), (1792, 256)])
            else:
                pieces.append([(0, 1024), (1024, 1024)])
        cc_ins, cc_outs = [], []
        for b in range(B):
            cc_ins.append([dram.tile([CH, n], fp16,
                                     name=f"ccin{b}_{t0}")
                           for (t0, n) in pieces[b]])
            cc_outs.append([dram.tile([n_cores * CH, n], fp16,
                                      addr_space="Shared",
                                      name=f"ccout{b}_{t0}")
                            for (t0, n) in pieces[b]])

        def pieces_of(b, t0, tlen):
            """(piece index, offset in piece, tok offset in [t0,t0+tlen),
            length) covering the token range"""
            out = []
            for i, (p0, n) in enumerate(pieces[b]):
                lo, hi = max(t0, p0), min(t0 + tlen, p0 + n)
                if lo < hi:
                    out.append((i, lo - p0, lo - t0, hi - lo))
            return out

        def issue_ag(b, i):
            nc.gpsimd.collective_compute(
                "AllGather",
                mybir.AluOpType.bypass,
                replica_groups=[list(range(n_cores))],
                ins=[cc_ins[b][i][:].opt()],
                outs=[cc_outs[b][i][:].opt()],
            )

        def proj_row_fillers(b):
            # output projection of batch b; one filler per AllGather-piece
            # slice of each 512-token row (so each group is gated only by
            # the piece it reads)
            def row(n, pi, off, qo, qn):
                def go():
                    rt = xin.tile([P, KT_E * qn], fp16, tag="xcol")
                    # issue on the ACT HWDGE queue so a wait on the
                    # AllGather doesn't head-of-line block the sync queue
                    nc.scalar.dma_start(
                        rt[:].rearrange("p (k q) -> p k q", k=KT_E),
                        cc_outs[b][pi][:, off:off + qn]
                        .rearrange("(k p) q -> p k q", p=P),
                    )
                    ps = stp.tile([P, 2 * NQ], f32, tag="st")
                    for k in range(KT_E):
                        nc.tensor.matmul(
                            ps[0:OC, 0:qn],
                            lhsT=wp_sb[:, k * OC:(k + 1) * OC],
                            rhs=rt[:, k * qn:(k + 1) * qn],
                            start=(k == 0),
                            stop=(k == KT_E - 1),
                        )
                    yo = evac.tile([OC, qn], f32, tag="yo")
                    nc.vector.tensor_scalar_add(yo[:], ps[0:OC, 0:qn],
                                                bp_sb[:, 0:1])
                    t0 = b * T + n * NQ + qo
                    nc.sync.dma_start(yT[:, t0:t0 + qn], yo[:])
                return go

            out = []
            for n in range(T // NQ):
                for (pi, off, qo, qn) in pieces_of(b, n * NQ, NQ):
                    out.append(row(n, pi, off, qo, qn))
            return out

        def attention(b, fillers):
            """kt-stream of batch b with PE filler groups interleaved."""
            fillers = list(fillers)
            # pace the fillers across the whole batch: later q-tiles have
            # the longest exp() stretches and need PE work the most
            ndrains = [sum(1 for kt in range(DPB * qt + DPB) if kt % 2 == 1)
                       + 1 for qt in range(TQ)]
            budget = [0]
            budget[0] = sum(ndrains)

            def drain(force=False):
                if not force:
                    budget[0] -= 1
                if fillers and (force or len(fillers) >= budget[0]):
                    f = fillers.pop(0)
                    if f is not None:
                        f()

            for qt in range(TQ):
                qs = b * T + qt * NQ
                nkt = DPB * qt + DPB
                pvs = [pvp.tile([P, NQ], f32, tag="pv", name=f"pv{_h}")
                       for _h in range(hpc)]
                pes = {}

                def issue_st(kt, qt=qt, qs=qs, pes=pes):
                    ks = b * T + kt * P
                    diag = kt >= DPB * qt
                    j = kt - DPB * qt
                    c0 = j * P if diag else 0
                    st = stp.tile([P, 2 * NQ], f32, tag="st")
                    for hh in range(hpc):
                        nc.tensor.matmul(
                            st[:, hh * NQ + c0:(hh + 1) * NQ],
                            lhsT=KTp[hh * hs:(hh + 1) * hs, ks:ks + P],
                            rhs=QT[hh * hs:(hh + 1) * hs, qs + c0:qs + NQ],
                            start=True,
                            stop=not diag,
                            tile_position=(hh * hs, 0),
                        )
                    if diag:
                        for hh in range(hpc):
                            nc.tensor.matmul(
                                st[:, hh * NQ + c0:hh * NQ + c0 + P],
                                lhsT=ident_t[:],
                                rhs=mask_sb[:],
                                start=False,
                                stop=True,
                            )
                    pe_t = pexp.tile([P, 2 * NQ], fp16, tag="pe")
                    nc.scalar.activation(
                        pe_t[:].rearrange("p (h q) -> p h q", h=hpc)
                        [:, :, c0:NQ],
                        st[:].rearrange("p (h q) -> p h q", h=hpc)
                        [:, :, c0:NQ],
                        Exp, scale=scale)
                    pes[kt] = (pe_t, c0)

                def issue_pv(kt, nkt=nkt, pvs=pvs, pes=pes):
                    vi = b * TK + kt
                    pe_t, c0 = pes.pop(kt)
                    for hh in range(hpc):
                        nc.tensor.matmul(
                            pvs[hh][0:hs + 1, c0:NQ],
                            lhsT=Vn[:, vi * WV + hh * (hs + 1):
                                    vi * WV + (hh + 1) * (hs + 1)],
                            rhs=pe_t[:, hh * NQ + c0:(hh + 1) * NQ],
                            start=(kt == 0),
                            stop=(kt == nkt - 1),
                        )

                for kt in range(nkt):
                    issue_st(kt)
                    if kt % 2 == 1:
                        drain()
                    if kt >= 2:
                        issue_pv(kt - 2)
                for kt in range(max(0, nkt - 2), nkt):
                    issue_pv(kt)

                # normalization: denominators -> broadcast -> reciprocal ->
                # scale; a filler between the DVE dens copy and the PE
                # broadcast hides the DVE latency
                dens = evac.tile([1, 2 * NQ], fp16, tag="den", bufs=3)
                for hh in range(hpc):
                    nc.vector.tensor_copy(dens[:, hh * NQ:(hh + 1) * NQ],
                                          pvs[hh][hs:hs + 1, :])
                drain()
                bc = stp.tile([P, 2 * NQ], f32, tag="st")
                for hh in range(hpc):
                    nc.tensor.matmul(
                        bc[0:hs, hh * NQ:(hh + 1) * NQ],
                        lhsT=ones_sb[0:1, :],
                        rhs=dens[:, hh * NQ:(hh + 1) * NQ],
                        start=True,
                        stop=True,
                    )
                recs = evac.tile([hs, 2 * NQ], f32, tag="rec", bufs=2)
                nc.vector.reciprocal_approx_fast(recs[:], bc[0:hs, :])
                aos = []
                for hh in range(hpc):
                    ao = evac.tile([hs, NQ], fp16, tag="ao")
                    nc.vector.tensor_mul(ao[:], pvs[hh][0:hs, :],
                                         recs[:, hh * NQ:(hh + 1) * NQ])
                    aos.append(ao)
                for (pi, off, qo, qn) in pieces_of(b, qt * NQ, NQ):
                    for hh in range(hpc):
                        nc.sync.dma_start(
                            cc_ins[b][pi][hh * hs:(hh + 1) * hs,
                                          off:off + qn],
                            aos[hh][:, qo:qo + qn],
                        )
                    p0, pn = pieces[b][pi]
                    if p0 + pn <= (qt + 1) * NQ:
                        issue_ag(b, pi)
            # leftover fillers (also give the last AG time to land before
            # the dependent projection rows run)
            while fillers:
                drain()

        # ---- main schedule ----
        row0 = qkv_row_fillers(0)
        row0[0]()
        load_late_consts()
        for g in row0[1:]:
            g()
        for g in qkv_row_fillers(1) + qkv_row_fillers(2) + qkv_row_fillers(3):
            g()
        proj_last = proj_row_fillers(B - 1)
        for b in range(B):
            if b + 1 < B:
                qkv = []
                for n in range(TQ):
                    qkv += qkv_row_fillers((b + 1) * TQ + n)
            else:
                qkv = [None] * 16
            # proj rows of b-1: rows 0,1 depend on the first AG piece of
            # b-1 (issued mid-attention(b-1), long done); rows 2,3 on the
            # second piece (issued at attention(b-1) end) -> place late
            proj = proj_row_fillers(b - 1) if b >= 1 else [None] * 4
            fillers = qkv[0:8] + proj[0:2] + qkv[8:16] + proj[2:4]
            if b + 1 == B:
                # last batch: slot its own first-piece projection rows at
                # the very end of the stream (their AG is issued at qt1)
                fillers += proj_last[0:2]
            attention(b, fillers)
        # tail: the remaining projection rows of the last batch, each
        # gated by its own AllGather piece
        for g in proj_last[2:]:
            g()

    nc.compile()
    return nc


def shard_inputs(x, W_qkv, b_qkv, W_proj, b_proj, H, n_cores):
    B, T, C = x.shape
    hs = C // H
    hpc = H // n_cores
    CH = hpc * hs
    OC = C // n_cores
    x2 = np.asarray(x, dtype=np.float32).reshape(B * T, C)
    xT = np.ascontiguousarray(x2.T.astype(np.float16))
    W_qkv = np.asarray(W_qkv, dtype=np.float32)
    b_qkv = np.asarray(b_qkv, dtype=np.float32)
    W_proj = np.asarray(W_proj, dtype=np.float32)
    b_proj = np.asarray(b_proj, dtype=np.float32)
    in_maps = []
    for i in range(n_cores):
        sl = slice(i * CH, (i + 1) * CH)
        wqkv_i = np.ascontiguousarray(np.concatenate(
            [W_qkv[:, sl], W_qkv[:, C:][:, sl], W_qkv[:, 2 * C:][:, sl]],
            axis=1).astype(np.float16))
        bqkv_i = np.ascontiguousarray(np.stack(
            [b_qkv[sl], b_qkv[C:][sl], b_qkv[2 * C:][sl]], axis=1))
        wp_i = np.ascontiguousarray(
            W_proj[:, i * OC:(i + 1) * OC].astype(np.float16))
        bp_i = np.ascontiguousarray(b_proj[i * OC:(i + 1) * OC].reshape(OC, 1))
        in_maps.append({"xT": xT, "wqkv": wqkv_i, "bqkv": bqkv_i,
                        "wp": wp_i, "bp": bp_i})
    return in_maps


def gather_output(results, B, T, C, n_cores):
    yT = np.concatenate([results[i]["yT"] for i in range(n_cores)], axis=0)
    return np.ascontiguousarray(yT.T).reshape(B, T, C).astype(np.float32)


_NC_CACHE = {}


def _get_nc(B, T, C, H, n_cores):
    key = (B, T, C, H, n_cores)
    if key not in _NC_CACHE:
        _NC_CACHE[key] = build_attention_nc(B, T, C, H, n_cores)
    return _NC_CACHE[key]


def kernel(x, W_qkv, b_qkv, W_proj, b_proj):
    from concourse import bass_utils

    B, T, C = 4, 2048, 1024
    H, n_cores = 16, 8
    assert x.shape == (B, T, C)
    nc = _get_nc(B, T, C, H, n_cores)
    in_maps = shard_inputs(x, W_qkv, b_qkv, W_proj, b_proj, H, n_cores)
    res = bass_utils.run_bass_kernel_spmd(
        nc, in_maps, core_ids=list(range(n_cores))
    )
    return gather_output(res.results, B, T, C, n_cores)


# revision 37
# speedup vs baseline: 1.0201x; 1.0201x over previous
"""Causal self-attention on 8 Trainium2 NeuronCores (Bass/Tile).

Problem: nn_CausalSelfAttention (B=4, T=2048, C=1024, H=16 heads, fp32).

Sharding: tensor-parallel over heads for QKV projection + attention
(2 heads per core), per-batch AllGather of attention outputs (fp16,
transposed layout), then tensor-parallel over output columns for the
final projection (each core computes a 128-column slice of x@W_proj).

Schedule: software-pipelined per batch.  The attention kt-stream of
batch b is interleaved with "PE filler" groups — the QKV projection
row-tiles of batch b+1 and the output-projection row-tiles of batch
b-1 — so the tensor engine never idles while the scalar engine works
through the exp() stream (exp is the per-batch critical path; PE idle
gaps would also drop the HAM clock gate to half rate).

Layouts (feature dim on partitions everywhere):
  xT      [C, B*T]        input (fp16), replicated to all cores
  Q^T,K^T [CH, B*T]       CH = 2 heads x 64, heads stacked on
                          partitions 0:64 / 64:128
  V       [B*T, CH]       matmul lhsT for P@V, stored tiled with an
                          extra ones-column per head so the PV matmul
                          also produces softmax denominators
  S^T     [kr, 2*q] pair  scores transposed, both heads side by side in
                          one 2-bank PSUM tile; the two score matmuls
                          are K=64 row-tiled pairs (tile_position
                          (0,0)/(64,0)) that execute concurrently; one
                          exp() activation covers both heads
  attn^T  [CH, T] fp16    per-core, per-batch -> AllGather -> [C, T]
  y^T     [OC, B*T]       per-core 128-column slice of the final output

Softmax: unnormalized exp (scores are O(1)); causal mask = PE add of a
-60000 upper-triangular [128,128] constant onto the diagonal strip;
denominator from the V ones-column; division via
reciprocal_approx_fast on the PE-broadcast denominator.
All matmuls run fp16 inputs with fp32 PSUM accumulation.
"""

import numpy as np
from contextlib import ExitStack

P = 128
NQ = 512  # q/moving-operand tile width
MASKVAL = -60000.0


def build_attention_nc(B, T, C, H, n_cores):
    import concourse.bass as bass  # noqa: F401
    import concourse.bacc as bacc
    import concourse.tile as tile
    import concourse.mybir as mybir

    f32 = mybir.dt.float32
    fp16 = mybir.dt.float16
    Exp = mybir.ActivationFunctionType.Exp

    hs = C // H              # head size
    hpc = H // n_cores       # heads per core
    CH = hpc * hs            # qkv channels per core
    OC = C // n_cores        # output columns per core
    NT = B * T               # tokens
    KT_E = C // P            # contraction tiles over embedding dim
    TQ = T // NQ             # q tiles per batch
    TK = T // P              # kr tiles per batch
    TKALL = NT // P          # kr tiles over all batches
    DPB = NQ // P            # kr-tiles crossing one q-tile's diagonal
    WV = hpc * (hs + 1)      # V storage width per kr-tile (with ones cols)

    assert T % NQ == 0 and C % P == 0 and NT % NQ == 0
    assert CH == P and H % n_cores == 0 and hpc == 2 and hs == 64
    scale = 1.0 / float(np.sqrt(hs))

    nc = bacc.Bacc("TRN2", target_bir_lowering=False, debug=False,
                   num_devices=n_cores)

    xT = nc.dram_tensor("xT", [C, NT], fp16, kind="ExternalInput")
    wqkv = nc.dram_tensor("wqkv", [C, 3 * CH], fp16, kind="ExternalInput")
    bqkv = nc.dram_tensor("bqkv", [CH, 3], f32, kind="ExternalInput")
    wp = nc.dram_tensor("wp", [C, OC], fp16, kind="ExternalInput")
    bp = nc.dram_tensor("bp", [OC, 1], f32, kind="ExternalInput")
    yT = nc.dram_tensor("yT", [OC, NT], f32, kind="ExternalOutput")

    ident_np = np.eye(P, dtype=np.float16)
    # mask[p, c] = MASKVAL where kr-offset p > q-offset c (strictly lower)
    mask_np = np.where(
        np.arange(P)[:, None] > np.arange(P)[None, :],
        np.float16(MASKVAL), np.float16(0.0)).astype(np.float16)
    ident_dram = nc.inline_tensor(ident_np, name="ident_const")
    mask_dram = nc.inline_tensor(mask_np, name="mask_const")
    ones_dram = nc.inline_tensor(np.ones((P, hs), dtype=np.float16),
                                 name="ones_const")
    # Vn image with the denominator ones-columns baked in; the value
    # columns are overwritten by the V transposes at runtime
    vinit_np = np.zeros((P, TKALL * WV), dtype=np.float16)
    vinit_np.reshape(P, TKALL, hpc, hs + 1)[:, :, :, hs] = 1.0
    vinit_dram = nc.inline_tensor(vinit_np, name="vinit_const")

    with tile.TileContext(nc) as tc, ExitStack() as ctx:
        const = ctx.enter_context(tc.tile_pool(name="const", bufs=1))
        big = ctx.enter_context(tc.tile_pool(name="big", bufs=1))
        xin = ctx.enter_context(tc.tile_pool(name="xin", bufs=7))
        evac = ctx.enter_context(tc.tile_pool(name="evac", bufs=3))
        pexp = ctx.enter_context(tc.tile_pool(name="pexp", bufs=5))
        stp = ctx.enter_context(tc.tile_pool(name="stp", bufs=3, space="PSUM"))
        pvp = ctx.enter_context(tc.tile_pool(name="pvp", bufs=2, space="PSUM"))
        dram = ctx.enter_context(tc.tile_pool(name="dram", bufs=1, space="DRAM"))

        ident_t = const.tile([P, P], fp16)
        mask_sb = const.tile([P, P], fp16)
        ones_sb = const.tile([P, hs], fp16)
        bqkv_sb = const.tile([CH, 3], f32)
        bp_sb = const.tile([OC, 1], f32)
        w_sb = const.tile([P, KT_E * 3 * CH], fp16)
        wp_sb = const.tile([P, KT_E * OC], fp16)

        nc.sync.dma_start(bqkv_sb[:], bqkv[:])
        nc.sync.dma_start(ident_t[:], ident_dram[:])
        nc.sync.dma_start(mask_sb[:], mask_dram[:])
        nc.sync.dma_start(ones_sb[:], ones_dram[:])
        nc.sync.dma_start(bp_sb[:], bp[:])
        # weights k-slice by k-slice so the first QKV matmul can start
        # ~1us after the first two transfers instead of after the full load
        for k in range(KT_E):
            nc.sync.dma_start(
                w_sb[:, k * 3 * CH:(k + 1) * 3 * CH],
                wqkv[k * P:(k + 1) * P, :],
            )

        QT = big.tile([P, NT], fp16)
        KTp = big.tile([P, NT], fp16)
        VT = big.tile([CH, NT], fp16)
        Vn = big.tile([P, TKALL * WV], fp16)

        def load_late_consts():
            # not needed until the attention-PV / projection phases
            # (ones columns of V via one contiguous DMA of the full Vn
            # image; value columns get overwritten later)
            nc.sync.dma_start(Vn[:], vinit_dram[:])
            nc.sync.dma_start(
                wp_sb[:].rearrange("p (k m) -> p k m", k=KT_E),
                wp[:].rearrange("(k p) m -> p k m", p=P),
            )

        # ---- QKV projection for one 512-token row-tile; emitted either
        # directly (batch 0) or as filler groups inside an attention batch.
        # The x loads are split out so they can be issued well before the
        # AllGathers start saturating HBM bandwidth.
        def qkv_row_loader(n):
            ns = n * NQ
            xt_box = []

            def load_x():
                xt = xin.tile([P, KT_E * NQ], fp16, tag="xcol")
                nc.sync.dma_start(
                    xt[:].rearrange("p (k q) -> p k q", k=KT_E),
                    xT[:, ns:ns + NQ].rearrange("(k p) q -> p k q", p=P),
                )
                xt_box.append(xt)

            return load_x, xt_box

        def qkv_row_fillers(n, xt_box):
            ns = n * NQ

            def mm_group(m):
                def go():
                    xt = xt_box[0]
                    ps = stp.tile([P, 2 * NQ], f32, tag="st")
                    for k in range(KT_E):
                        nc.tensor.matmul(
                            ps[:, 0:NQ],
                            lhsT=w_sb[:, k * 3 * CH + m * CH:
                                      k * 3 * CH + (m + 1) * CH],
                            rhs=xt[:, k * NQ:(k + 1) * NQ],
                            start=(k == 0),
                            stop=(k == KT_E - 1),
                        )
                    dst = (QT, KTp, VT)[m]
                    nc.vector.tensor_scalar_add(dst[:, ns:ns + NQ],
                                                ps[:, 0:NQ],
                                                bqkv_sb[:, m:m + 1])
                return go

            def transposes():
                tp = stp.tile([P, DPB * CH], fp16, tag="st", name="tp")
                for j in range(DPB):
                    nc.tensor.transpose(
                        tp[:, j * CH:(j + 1) * CH],
                        VT[:, (n * DPB + j) * P:(n * DPB + j + 1) * P],
                        ident_t[:],
                    )
                vi0 = n * DPB
                dst = Vn[:, vi0 * WV:(vi0 + DPB) * WV].rearrange(
                    "p (v h d) -> p v h d", h=hpc, d=hs + 1
                )[:, :, :, 0:hs]
                nc.vector.tensor_copy(dst, tp[:].rearrange(
                    "p (v h d) -> p v h d", h=hpc, d=hs))

            return [mm_group(0), mm_group(1), mm_group(2), transposes]

        # ---- per-batch AllGather pieces (token ranges): every batch is
        # gathered in two halves so the projection rows unblock early; the
        # last batch's tail is split ever finer so the final AllGather ->
        # projection chain is as short as possible
        pieces = []  # per batch: list of (tok_start, tok_len)
        for b in range(B):
            if b == B - 1:
                pieces.append([(0, 1024), (1024, 512),
                               (1536, 256), (1792, 256)])
            else:
                pieces.append([(0, 1024), (1024, 1024)])
        cc_ins, cc_outs = [], []
        for b in range(B):
            cc_ins.append([dram.tile([CH, n], fp16,
                                     name=f"ccin{b}_{t0}")
                           for (t0, n) in pieces[b]])
            cc_outs.append([dram.tile([n_cores * CH, n], fp16,
                                      addr_space="Shared",
                                      name=f"ccout{b}_{t0}")
                            for (t0, n) in pieces[b]])

        def pieces_of(b, t0, tlen):
            """(piece index, offset in piece, tok offset in [t0,t0+tlen),
            length) covering the token range"""
            out = []
            for i, (p0, n) in enumerate(pieces[b]):
                lo, hi = max(t0, p0), min(t0 + tlen, p0 + n)
                if lo < hi:
                    out.append((i, lo - p0, lo - t0, hi - lo))
            return out

        def issue_ag(b, i):
            nc.gpsimd.collective_compute(
                "AllGather",
                mybir.AluOpType.bypass,
                replica_groups=[list(range(n_cores))],
                ins=[cc_ins[b][i][:].opt()],
                outs=[cc_outs[b][i][:].opt()],
            )

        def proj_row_fillers(b):
            # output projection of batch b; one filler per AllGather-piece
            # slice of each 512-token row (so each group is gated only by
            # the piece it reads)
            def row(n, pi, off, qo, qn):
                def go():
                    rt = xin.tile([P, KT_E * qn], fp16, tag="xcol")
                    # issue on the ACT HWDGE queue so a wait on the
                    # AllGather doesn't head-of-line block the sync queue
                    nc.scalar.dma_start(
                        rt[:].rearrange("p (k q) -> p k q", k=KT_E),
                        cc_outs[b][pi][:, off:off + qn]
                        .rearrange("(k p) q -> p k q", p=P),
                    )
                    ps = stp.tile([P, 2 * NQ], f32, tag="st")
                    for k in range(KT_E):
                        nc.tensor.matmul(
                            ps[0:OC, 0:qn],
                            lhsT=wp_sb[:, k * OC:(k + 1) * OC],
                            rhs=rt[:, k * qn:(k + 1) * qn],
                            start=(k == 0),
                            stop=(k == KT_E - 1),
                        )
                    yo = evac.tile([OC, qn], f32, tag="yo")
                    nc.vector.tensor_scalar_add(yo[:], ps[0:OC, 0:qn],
                                                bp_sb[:, 0:1])
                    t0 = b * T + n * NQ + qo
                    nc.sync.dma_start(yT[:, t0:t0 + qn], yo[:])
                return go

            out = []
            for n in range(T // NQ):
                for (pi, off, qo, qn) in pieces_of(b, n * NQ, NQ):
                    out.append(row(n, pi, off, qo, qn))
            return out

        def attention(b, fillers, prefetch=None):
            """kt-stream of batch b with PE filler groups interleaved."""
            fillers = list(fillers)
            # pace the fillers across the whole batch: later q-tiles have
            # the longest exp() stretches and need PE work the most
            ndrains = [sum(1 for kt in range(DPB * qt + DPB) if kt % 2 == 1)
                       + 1 for qt in range(TQ)]
            budget = [0]
            budget[0] = sum(ndrains)

            def drain(force=False):
                if not force:
                    budget[0] -= 1
                if fillers and (force or len(fillers) >= budget[0]):
                    f = fillers.pop(0)
                    if f is not None:
                        f()

            for qt in range(TQ):
                if qt == TQ - 1 and prefetch is not None:
                    prefetch()
                qs = b * T + qt * NQ
                nkt = DPB * qt + DPB
                pvs = [pvp.tile([P, NQ], f32, tag="pv", name=f"pv{_h}")
                       for _h in range(hpc)]
                pes = {}

                def issue_st(kt, qt=qt, qs=qs, pes=pes):
                    ks = b * T + kt * P
                    diag = kt >= DPB * qt
                    j = kt - DPB * qt
                    c0 = j * P if diag else 0
                    st = stp.tile([P, 2 * NQ], f32, tag="st")
                    for hh in range(hpc):
                        nc.tensor.matmul(
                            st[:, hh * NQ + c0:(hh + 1) * NQ],
                            lhsT=KTp[hh * hs:(hh + 1) * hs, ks:ks + P],
                            rhs=QT[hh * hs:(hh + 1) * hs, qs + c0:qs + NQ],
                            start=True,
                            stop=not diag,
                            tile_position=(hh * hs, 0),
                        )
                    if diag:
                        for hh in range(hpc):
                            nc.tensor.matmul(
                                st[:, hh * NQ + c0:hh * NQ + c0 + P],
                                lhsT=ident_t[:],
                                rhs=mask_sb[:],
                                start=False,
                                stop=True,
                            )
                    pe_t = pexp.tile([P, 2 * NQ], fp16, tag="pe")
                    nc.scalar.activation(
                        pe_t[:].rearrange("p (h q) -> p h q", h=hpc)
                        [:, :, c0:NQ],
                        st[:].rearrange("p (h q) -> p h q", h=hpc)
                        [:, :, c0:NQ],
                        Exp, scale=scale)
                    pes[kt] = (pe_t, c0)

                def issue_pv(kt, nkt=nkt, pvs=pvs, pes=pes):
                    vi = b * TK + kt
                    pe_t, c0 = pes.pop(kt)
                    for hh in range(hpc):
                        nc.tensor.matmul(
                            pvs[hh][0:hs + 1, c0:NQ],
                            lhsT=Vn[:, vi * WV + hh * (hs + 1):
                                    vi * WV + (hh + 1) * (hs + 1)],
                            rhs=pe_t[:, hh * NQ + c0:(hh + 1) * NQ],
                            start=(kt == 0),
                            stop=(kt == nkt - 1),
                        )

                for kt in range(nkt):
                    issue_st(kt)
                    if kt % 2 == 1:
                        drain()
                    if kt >= 2:
                        issue_pv(kt - 2)
                for kt in range(max(0, nkt - 2), nkt):
                    issue_pv(kt)

                # normalization: denominators -> broadcast -> reciprocal ->
                # scale; a filler between the DVE dens copy and the PE
                # broadcast hides the DVE latency
                dens = evac.tile([1, 2 * NQ], fp16, tag="den", bufs=3)
                for hh in range(hpc):
                    nc.vector.tensor_copy(dens[:, hh * NQ:(hh + 1) * NQ],
                                          pvs[hh][hs:hs + 1, :])
                drain()
                bc = stp.tile([P, 2 * NQ], f32, tag="st")
                for hh in range(hpc):
                    nc.tensor.matmul(
                        bc[0:hs, hh * NQ:(hh + 1) * NQ],
                        lhsT=ones_sb[0:1, :],
                        rhs=dens[:, hh * NQ:(hh + 1) * NQ],
                        start=True,
                        stop=True,
                    )
                recs = evac.tile([hs, 2 * NQ], f32, tag="rec", bufs=2)
                nc.vector.reciprocal_approx_fast(recs[:], bc[0:hs, :])
                aos = []
                for hh in range(hpc):
                    ao = evac.tile([hs, NQ], fp16, tag="ao")
                    nc.vector.tensor_mul(ao[:], pvs[hh][0:hs, :],
                                         recs[:, hh * NQ:(hh + 1) * NQ])
                    aos.append(ao)
                for (pi, off, qo, qn) in pieces_of(b, qt * NQ, NQ):
                    for hh in range(hpc):
                        nc.sync.dma_start(
                            cc_ins[b][pi][hh * hs:(hh + 1) * hs,
                                          off:off + qn],
                            aos[hh][:, qo:qo + qn],
                        )
                    p0, pn = pieces[b][pi]
                    if p0 + pn <= (qt + 1) * NQ:
                        issue_ag(b, pi)
            # leftover fillers (also give the last AG time to land before
            # the dependent projection rows run)
            while fillers:
                drain()

        # ---- main schedule ----
        loaders = [qkv_row_loader(n) for n in range(B * TQ)]
        for n in range(TQ):
            loaders[n][0]()
        row0 = qkv_row_fillers(0, loaders[0][1])
        row0[0]()
        load_late_consts()
        for g in row0[1:]:
            g()
        for n in range(1, TQ):
            for g in qkv_row_fillers(n, loaders[n][1]):
                g()
        # batch 1's x rows load during the QKV(0) compute, ahead of the
        # first AllGather's HBM traffic
        for n in range(TQ, 2 * TQ):
            loaders[n][0]()
        proj_last = proj_row_fillers(B - 1)
        for b in range(B):
            if b + 1 < B:
                qkv = []
                for n in range(TQ):
                    r = (b + 1) * TQ + n
                    qkv += qkv_row_fillers(r, loaders[r][1])
            else:
                qkv = [None] * 16
            # proj rows of b-1: rows 0,1 depend on the first AG piece of
            # b-1 (issued mid-attention(b-1), long done); rows 2,3 on the
            # second piece (issued at attention(b-1) end) -> place late
            proj = proj_row_fillers(b - 1) if b >= 1 else [None] * 4
            fillers = qkv[0:8] + proj[0:2] + qkv[8:16] + proj[2:4]
            if b + 1 == B:
                # last batch: slot its own first-piece projection rows at
                # the very end of the stream (their AG is issued at qt1)
                fillers += proj_last[0:2]

            def prefetch(b=b):
                # x rows 0,1 of batch b+2, ahead of the b+1 AllGathers
                if b + 2 < B:
                    for n in range((b + 2) * TQ, (b + 2) * TQ + 2):
                        loaders[n][0]()

            attention(b, fillers, prefetch)
            # x rows 2,3 of batch b+2 (needed only late in attention(b+1))
            if b + 2 < B:
                for n in range((b + 2) * TQ + 2, (b + 3) * TQ):
                    loaders[n][0]()
        # tail: the remaining projection rows of the last batch, each
        # gated by its own AllGather piece
        for g in proj_last[2:]:
            g()

    nc.compile()
    return nc


def shard_inputs(x, W_qkv, b_qkv, W_proj, b_proj, H, n_cores):
    B, T, C = x.shape
    hs = C // H
    hpc = H // n_cores
    CH = hpc * hs
    OC = C // n_cores
    x2 = np.asarray(x, dtype=np.float32).reshape(B * T, C)
    xT = np.ascontiguousarray(x2.T.astype(np.float16))
    W_qkv = np.asarray(W_qkv, dtype=np.float32)
    b_qkv = np.asarray(b_qkv, dtype=np.float32)
    W_proj = np.asarray(W_proj, dtype=np.float32)
    b_proj = np.asarray(b_proj, dtype=np.float32)
    in_maps = []
    for i in range(n_cores):
        sl = slice(i * CH, (i + 1) * CH)
        wqkv_i = np.ascontiguousarray(np.concatenate(
            [W_qkv[:, sl], W_qkv[:, C:][:, sl], W_qkv[:, 2 * C:][:, sl]],
            axis=1).astype(np.float16))
        bqkv_i = np.ascontiguousarray(np.stack(
            [b_qkv[sl], b_qkv[C:][sl], b_qkv[2 * C:][sl]], axis=1))
        wp_i = np.ascontiguousarray(
            W_proj[:, i * OC:(i + 1) * OC].astype(np.float16))
        bp_i = np.ascontiguousarray(b_proj[i * OC:(i + 1) * OC].reshape(OC, 1))
        in_maps.append({"xT": xT, "wqkv": wqkv_i, "bqkv": bqkv_i,
                        "wp": wp_i, "bp": bp_i})
    return in_maps


def gather_output(results, B, T, C, n_cores):
    yT = np.concatenate([results[i]["yT"] for i in range(n_cores)], axis=0)
    return np.ascontiguousarray(yT.T).reshape(B, T, C).astype(np.float32)


_NC_CACHE = {}


def _get_nc(B, T, C, H, n_cores):
    key = (B, T, C, H, n_cores)
    if key not in _NC_CACHE:
        _NC_CACHE[key] = build_attention_nc(B, T, C, H, n_cores)
    return _NC_CACHE[key]


def kernel(x, W_qkv, b_qkv, W_proj, b_proj):
    from concourse import bass_utils

    B, T, C = 4, 2048, 1024
    H, n_cores = 16, 8
    assert x.shape == (B, T, C)
    nc = _get_nc(B, T, C, H, n_cores)
    in_maps = shard_inputs(x, W_qkv, b_qkv, W_proj, b_proj, H, n_cores)
    res = bass_utils.run_bass_kernel_spmd(
        nc, in_maps, core_ids=list(range(n_cores))
    )
    return gather_output(res.results, B, T, C, n_cores)
